# revision 35
# baseline (speedup 1.0000x reference)
"""Trainium2 Bass kernel for nn_CustomNetworkGINSeroMean (GIN message passing +
TopK pooling + SERO readout + BN/FC head).

Strategy (data-parallel over batch B=64, 8 graphs per NeuronCore):
  - All per-graph state lives in SBUF; inputs arrive as a handful of large
    packed DMAs (consts in one [128,W] tensor; x/adj/adjT/posT packed
    [100, 8*w] graph-major) to kill the descriptor storm.
  - Host pre-transposes adj and pos, so no on-chip transposes are needed for
    the mean aggregation: y|deg = adjT.T @ [x | alive] in one PE matmul
    (deg = sum_j adj_ij * alive_j rides along as an extra rhs column), then
    h = x + invd * y in one DVE scalar_tensor_tensor.
  - TopK pooling is dense with alive-masks (DVE max8/match_replace idiom).
  - Wn = (relu(pos@w1)@w2+b2) never materialized: x_out = sum_k U_k*(h@W2r[k])
    with U_8 = 1 folding b2 in; the k-sum is split DVE (k<4, mult+reduce) /
    Pool (k>=4, stt accumulation chain) to balance engines.
  - augment_adj keeps both A and A^T resident: half-masked operands make the
    contraction fully alive-masked (alive_j on both sides), M^2 = G + 2*Abar
    (+diag, killed by notI); dead-row/col pollution never reaches live math.
  - SERO readout: r gathered cross-core as [8, 3H] rows so the AllGather
    output is one contiguous [64, 192] load; BN affine+gelu fused into one
    scalar-engine activation (bias/scale per-partition operands).
"""

import numpy as np

import concourse.bass as bass
import concourse.tile as tile
from concourse import bacc, mybir
from concourse.bass import ts
from concourse.bass_utils import run_bass_kernel_spmd
from concourse.masks import make_identity

F32 = mybir.dt.float32
F32R = mybir.dt.float32r
AF = mybir.ActivationFunctionType
ALU = mybir.AluOpType
AX = mybir.AxisListType

B, R, D = 64, 100, 100
H = 64
K = 8
KE = K + 1
HID = (H, H, H)
IN = (D, H, H)
FC = (64, 32)
NCLASS = 2
N_LAYERS = 3
NCORES = 8
BL = B // NCORES
MS = (50, 25, 13)
NEG = -1.0e30
EPS_BN = 1e-5
GW = R + 1  # per-graph x block width (feats + alive col)

# const pack column offsets
C_ID, C_NOTI, C_PW, C_W1 = 0, 128, 256, 448
C_SEW, C_SAW, C_FCW0, C_FCW1 = 472, 664, 856, 1048
C_FW, C_ONES, C_SAB, C_SBG, C_SBB = 1080, 1082, 1083, 1086, 1089
C_FCB0, C_BNG0, C_BNB0, C_FCB1, C_BNG1, C_BNB1, C_FB = 1092, 1093, 1094, 1095, 1096, 1097, 1098
C_EPS = 1099
C_ID2 = 1100
C_W = 1228

TRACE = False
_CACHE = {}


def _emit(tc, io):
    nc = tc.nc
    consts = io["consts_pool"]
    state = io["state_pool"]
    work = io["work_pool"]
    psum = io["psum_pool"]
    dram = io["dram_pool"]

    cp = consts.tile([128, C_W], F32, tag="cpack")
    nc.sync.dma_start(cp[:], io["cpack"][:])
    w2f = consts.tile([128, 3 * KE * H], F32, tag="w2packf")
    nc.sync.dma_start(w2f[:], io["w2pack"][:])
    w2 = consts.tile([128, 3 * KE * H], F32R, tag="w2pack")
    nc.vector.tensor_copy(w2[:], w2f[:])
    id2r = consts.tile([128, 128], F32R, tag="id2r")
    nc.vector.tensor_copy(id2r[:], cp[:, C_ID2 : C_ID2 + 128])

    xall = state.tile([R, BL * GW], F32, tag="xall")
    nc.sync.dma_start(xall[:], io["xpack"][:])
    adjall = state.tile([R, BL * R], F32, tag="adjall")
    nc.sync.dma_start(adjall[:], io["adjpack"][:])
    adjTall = state.tile([R, BL * R], F32, tag="adjTall")
    nc.sync.dma_start(adjTall[:], io["adjTpack"][:])
    posTall = state.tile([R, BL * R], F32, tag="posTall")
    nc.sync.dma_start(posTall[:], io["posTpack"][:])

    ident = cp[:, C_ID : C_ID + 128]
    notI = cp[:, C_NOTI : C_NOTI + 128]
    ones_col = cp[:, C_ONES : C_ONES + 1]

    # U = relu(pos @ w1) depends only on static pos: compute ALL layers up
    # front (overlaps input DMA / init barrier). Layout [li][g*KE + k], col
    # k=K stays 1.0 to fold the b2 bias in as a 9th term.
    ueall = state.tile([R, N_LAYERS * BL * KE], F32, tag="ueall")
    nc.vector.memset(ueall[:], 1.0)
    for li in range(N_LAYERS):
        ub_ps = psum.tile([R, BL * K], F32, tag="sm")
        for g in range(BL):
            nc.tensor.matmul(
                ub_ps[:, g * K : (g + 1) * K],
                posTall[:, g * R : (g + 1) * R],
                cp[:D, C_W1 + li * K : C_W1 + (li + 1) * K],
            )
        uv = ueall[:, li * BL * KE :].rearrange("n (g e) -> n g e", e=KE)
        nc.scalar.activation(
            uv[:, 0:BL, 0:K], ub_ps[:].rearrange("n (g k) -> n g k", k=K), AF.Relu
        )

    xoall = state.tile([R, BL * H], F32, tag="xoall")
    rlocT = state.tile([BL, N_LAYERS * H], F32, tag="rlocT")


    import os
    ksub = int(os.environ.get("KSUB", "4"))
    kph = int(os.environ.get("KPH", "4"))
    KA = 5  # g1 holds k in [0, KA), g2 holds [KA, KE)
    aliveT = None  # [BL, R] graph-major; None == all alive (layer 0)
    alCol = None

    din = D
    for li in range(N_LAYERS):
        hid = HID[li]
        m_sel = MS[li]
        scoreCol = work.tile([R, BL], F32, tag="scoreCol")

        # ---- batched degrees: deg_g = adjT_g.T @ alive, one reciprocal ----
        deg_ps = psum.tile([R, BL], F32, tag="sm")
        for g in range(BL):
            nc.tensor.matmul(
                deg_ps[:, g : g + 1], adjTall[:, g * R : (g + 1) * R],
                ones_col[:R, :] if alCol is None else alCol[:, g : g + 1],
            )
        invds = work.tile([R, BL], F32, tag="invds")
        nc.vector.reciprocal(invds[:], deg_ps[:])

        # ---- mean aggregation, batched: h = x + invd * (adj @ x) ----
        # y matmuls land in shared PSUM banks (GB graphs per bank), then two
        # broadcast-view DVE ops normalize+add for the whole block at once.
        GB = 4 if din > 64 else 8
        xv = xall[:].rearrange("n (g w) -> n g w", w=GW)
        for b0 in range(0, BL, GB):
            yb_ps = psum.tile([R, GB * din], F32, tag="mm")
            for g in range(b0, b0 + GB):
                nc.tensor.matmul(
                    yb_ps[:, (g - b0) * din : (g - b0 + 1) * din],
                    adjTall[:, g * R : (g + 1) * R], xall[:, g * GW : g * GW + din],
                )
            yv = yb_ps[:].rearrange("n (g e) -> n g e", e=din)
            tb = work.tile([R, GB, din], F32, tag="tb")
            iv = invds[:, b0 : b0 + GB].unsqueeze(2).broadcast_to([R, GB, din])
            nc.vector.tensor_tensor(tb[:], yv, iv, ALU.mult)
            nc.vector.tensor_tensor(
                xv[:, b0 : b0 + GB, 0:din], tb[:], xv[:, b0 : b0 + GB, 0:din],
                ALU.add,
            )
        if kph >= 2:
            # hT = x.T per 4-graph PSUM bank, cast to f32r in two wide copies
            hTall = work.tile([R, BL * R], F32R, tag="hTall")
            for b0 in range(0, BL, 4):
                htb_ps = psum.tile([din, 4 * R], F32, tag="tp")
                for g in range(b0, b0 + 4):
                    nc.tensor.transpose(
                        htb_ps[:, (g - b0) * R : (g - b0 + 1) * R],
                        xall[:, g * GW : g * GW + din], ident[:R, :R],
                    )
                nc.vector.tensor_copy(
                    hTall[0:din, b0 * R : (b0 + 4) * R], htb_ps[:]
                )
            w2r = w2[0:din, li * KE * H : (li + 1) * KE * H]
            for g in range(BL):
                # ---- x_out = sum_k U_k * (h @ W2r[k]) ----
                hT = hTall[0:din, g * R : (g + 1) * R]
                ue = ueall[:, (li * BL + g) * KE : (li * BL + g + 1) * KE]
                g1_ps = psum.tile([R, KA * hid], F32, tag="mm")
                g2_ps = psum.tile([R, (KE - KA) * hid], F32, tag="mm2")
                nc.tensor.matmul(g1_ps[:], hT, w2r[:, 0 : KA * hid])
                nc.tensor.matmul(g2_ps[:], hT, w2r[:, KA * hid : KE * hid])
                if kph < 3:
                    continue
                # DVE: mult+reduce both halves
                pr1 = work.tile([R, hid, KA], F32, tag="pr1")
                g1v = g1_ps[:].rearrange("n (k o) -> n o k", k=KA)
                u1v = ue[:, 0:KA].unsqueeze(1).broadcast_to([R, hid, KA])
                nc.vector.tensor_tensor(pr1[:], g1v, u1v, ALU.mult)
                xo1 = work.tile([R, hid], F32, tag="xo1")
                nc.vector.tensor_reduce(xo1[:], pr1[:], AX.X, ALU.add)
                pr2 = work.tile([R, hid, KE - KA], F32, tag="pr2")
                g2v = g2_ps[:].rearrange("n (k o) -> n o k", k=KE - KA)
                u2v = ue[:, KA:KE].unsqueeze(1).broadcast_to([R, hid, KE - KA])
                nc.vector.tensor_tensor(pr2[:], g2v, u2v, ALU.mult)
                xo2 = work.tile([R, hid], F32, tag="xo2")
                nc.vector.tensor_reduce(xo2[:], pr2[:], AX.X, ALU.add)
                xo = xoall[:, g * H : g * H + hid]
                nc.gpsimd.tensor_tensor(xo, xo1[:], xo2[:], ALU.add)
        if kph >= 4:
            # ---- topk scores, batched: one mult + one strided reduce ----
            sprodall = work.tile([R, BL, hid], F32, tag="sprodall")
            pwb = (
                cp[:R, C_PW + li * H : C_PW + (li + 1) * H]
                .unsqueeze(1).broadcast_to([R, BL, hid])
            )
            nc.vector.tensor_tensor(
                sprodall[:], xoall[:].rearrange("n (g o) -> n g o", o=H), pwb,
                ALU.mult,
            )
            nc.vector.tensor_reduce(scoreCol[:], sprodall[:], AX.X, ALU.add)

        if ksub < 2:
            din = hid
            continue
        # ---- topk (graph-major [BL, R]) ----
        st_ps = psum.tile([BL, R], F32, tag="sm")
        nc.tensor.transpose(st_ps[:], scoreCol[:], ident[:R, :R])
        sm = work.tile([BL, R], F32, tag="smask")
        if aliveT is None:
            nc.vector.tensor_copy(sm[:], st_ps[:])
        else:
            pen = work.tile([BL, R], F32, tag="pen")
            nc.gpsimd.tensor_scalar(pen[:], aliveT[:], -1.0, -NEG, ALU.add, ALU.mult)
            nc.vector.tensor_tensor(sm[:], st_ps[:], aliveT[:], ALU.mult)
            nc.vector.tensor_tensor(sm[:], sm[:], pen[:], ALU.add)
        wk = work.tile([BL, R], F32, tag="wk")
        nc.vector.tensor_copy(wk[:], sm[:])
        nrounds = (m_sel + 7) // 8
        for t in range(nrounds):
            mx = work.tile([BL, 8], F32, tag="mx")
            nc.vector.max(mx[:], wk[:])
            rem = m_sel - 8 * t
            if rem < 8:
                nc.vector.memset(mx[:, rem:8], NEG)
            nc.vector.match_replace(wk[:], mx[:], wk[:], NEG)
        nmT = work.tile([BL, R], F32, tag=f"nmT{li}")
        nc.vector.tensor_tensor(nmT[:], sm[:], wk[:], ALU.subtract)
        nc.vector.tensor_scalar_min(nmT[:], nmT[:], 1.0)
        sig = work.tile([BL, R], F32, tag="sig")
        nc.scalar.activation(sig[:], sm[:], AF.Sigmoid)
        sclT = work.tile([BL, R], F32, tag="sclT")
        nc.vector.tensor_tensor(sclT[:], sig[:], nmT[:], ALU.mult)
        aliveT = nmT
        sc_ps = psum.tile([R, BL], F32, tag="tp")
        nc.tensor.transpose(sc_ps[:], sclT[:], ident[:BL, :BL])
        scalesCol = work.tile([R, BL], F32, tag=f"scales{li}")
        nc.vector.tensor_copy(scalesCol[:], sc_ps[:])
        alCol = work.tile([R, BL], F32, tag=f"alCol{li}")
        nc.gpsimd.tensor_scalar(alCol[:], scalesCol[:], 0.0, None, ALU.is_gt)

        if ksub < 3:
            din = hid
            continue
        # ---- pool x (batched), readout r, augment adj ----
        scb = scalesCol[:].unsqueeze(2).broadcast_to([R, BL, hid])
        nc.vector.tensor_tensor(
            xv[:, :, 0:hid], xoall[:].rearrange("n (g o) -> n g o", o=H), scb,
            ALU.mult,
        )
        rt_ps = psum.tile([H, BL], F32, tag="sm")
        for g in range(BL):
            nc.tensor.matmul(
                rt_ps[:, g : g + 1], xall[:, g * GW : g * GW + hid], ones_col[:R, :]
            )
        rtsb = work.tile([H, BL], F32, tag="rtsb")
        nc.scalar.mul(rtsb[:], rt_ps[:], 1.0 / m_sel)
        rlt_ps = psum.tile([BL, H], F32, tag="tp")
        nc.tensor.transpose(rlt_ps[:], rtsb[:], ident[:H, :H])
        nc.scalar.copy(rlocT[:, ts(li, H)], rlt_ps[:])
        if li == 1:
            rloc01 = dram.tile([BL, 2 * H], F32, tag="rloc01")
            nc.sync.dma_start(rloc01[:], rlocT[:, 0 : 2 * H])
            rg01 = dram.tile([NCORES, BL, 2 * H], F32, tag="rg01")
            nc.gpsimd.collective_compute(
                "AllGather",
                ALU.bypass,
                replica_groups=[list(range(NCORES))],
                ins=[rloc01[:].opt()],
                outs=[rg01[:].opt()],
            )
            io["rg01"] = rg01

        if li < N_LAYERS - 1 and ksub >= 4:
            # masks batched over all graphs via broadcast views
            alb = alCol[:].unsqueeze(2).broadcast_to([R, BL, R])
            adv = adjall[:].rearrange("n (g j) -> n g j", j=R)
            atv = adjTall[:].rearrange("n (g j) -> n g j", j=R)
            ahall = work.tile([R, BL * R], F32R, tag="ahall")
            nc.vector.tensor_tensor(
                ahall[:].rearrange("n (g j) -> n g j", j=R), adv, alb, ALU.mult
            )
            athall = work.tile([R, BL * R], F32R, tag="athall")
            nc.gpsimd.tensor_tensor(
                athall[:].rearrange("n (g j) -> n g j", j=R), atv, alb, ALU.mult
            )
            for g in range(BL):
                a0 = g * R
                G_ps = psum.tile([R, R], F32, tag="mm")
                nc.tensor.matmul(
                    G_ps[:], athall[:, a0 : a0 + R], ahall[:, a0 : a0 + R],
                    start=True, stop=False,
                )
                nc.tensor.matmul(
                    G_ps[:], id2r[:R, :R], ahall[:, a0 : a0 + R],
                    start=False, stop=True,
                )
                nc.vector.tensor_tensor(
                    adjall[:, a0 : a0 + R], G_ps[:], notI[:R, :R], ALU.mult
                )
                at_ps = psum.tile([R, R], F32, tag="mm2")
                nc.tensor.transpose(at_ps[:], adjall[:, a0 : a0 + R], ident[:R, :R])
                nc.scalar.copy(adjTall[:, a0 : a0 + R], at_ps[:])
        din = hid

    import os
    stage = int(os.environ.get("KSTAGE", "3"))
    if stage == 1:
        ofin = work.tile([B, NCLASS], F32, tag="ofin")
        nc.vector.memset(ofin[:], 0.0)
        if kph >= 4:
            nc.vector.tensor_tensor(ofin[:, 0:1], xoall[0:B, 0:1], scoreCol[0:B, 0:1], ALU.add)
        elif kph >= 3:
            nc.scalar.copy(ofin[:, 0:1], xoall[0:B, 0:1])
        nc.sync.dma_start(io["out"][:], ofin[:])
        return

    # ---- layer-2 AllGather is issued AFTER the (0,1) SERO block is
    # emitted, so the collective's sync-queue fence cannot stall the
    # pre-head work that only needs the first gather's result.
    rloc2 = dram.tile([BL, H], F32, tag="rloc2")
    nc.sync.dma_start(rloc2[:], rlocT[:, 2 * H : 3 * H])
    rg01 = io["rg01"]
    rgm = state.tile([B, N_LAYERS * H], F32, tag="rgm")
    nc.sync.dma_start(
        rgm[:].rearrange("cl (l h) -> cl l h", h=H)[:, 0:2, :],
        rg01[:].rearrange("c l (q h) -> (c l) q h", h=H),
    )
    if stage == 2:
        ofin = work.tile([B, NCLASS], F32, tag="ofin")
        nc.vector.memset(ofin[:], 0.0)
        nc.scalar.copy(ofin[:, 0:2], rgm[:, 0:2])
        nc.sync.dma_start(io["out"][:], ofin[:])
        return

    # ---- SERO attention per gather-group: layers (0,1) run while the
    # layer-2 AllGather is still in flight; layer 2 afterward.
    rcat = state.tile([H, N_LAYERS * B], F32, tag="rcat")
    serocat = state.tile([H, N_LAYERS * B], F32, tag="serocat")

    def sero_block(lis):
        nl = len(lis)
        l0 = lis[0]
        for li in lis:
            rt_ps2 = psum.tile([H, B], F32, tag="tp")
            nc.tensor.transpose(rt_ps2[:], rgm[:, li * H : (li + 1) * H], ident[:B, :B])
            nc.scalar.copy(rcat[:, li * B : (li + 1) * B], rt_ps2[:])
        LBn = nl * B
        z1_ps = psum.tile([H, LBn], F32, tag="mm")
        for j, li in enumerate(lis):
            nc.tensor.matmul(
                z1_ps[:, j * B : (j + 1) * B],
                cp[:H, C_SEW + li * H : C_SEW + (li + 1) * H],
                rcat[:, li * B : (li + 1) * B],
            )
        mus = work.tile([H, nl], F32, tag=f"mus{l0}")
        nc.vector.tensor_reduce(mus[:], z1_ps[:].rearrange("h (l b) -> h l b", b=B), AX.X, ALU.add)
        z1sb = work.tile([H, LBn], F32, tag=f"z1sb{l0}")
        nc.scalar.copy(z1sb[:], z1_ps[:])
        sqs = work.tile([H, LBn], F32, tag=f"sqs{l0}")
        nc.vector.tensor_tensor(sqs[:], z1sb[:], z1sb[:], ALU.mult)
        ssq = work.tile([H, nl], F32, tag=f"ssq{l0}")
        nc.vector.tensor_reduce(ssq[:], sqs[:].rearrange("h (l b) -> h l b", b=B), AX.X, ALU.add)
        mu3 = work.tile([H, nl], F32, tag=f"mu{l0}")
        nc.vector.tensor_scalar_mul(mu3[:], mus[:], 1.0 / B)
        musq = work.tile([H, nl], F32, tag=f"musq{l0}")
        nc.vector.tensor_tensor(musq[:], mu3[:], mu3[:], ALU.mult)
        var3 = work.tile([H, nl], F32, tag=f"var{l0}")
        nc.vector.scalar_tensor_tensor(
            var3[:], ssq[:], 1.0 / B, musq[:], ALU.mult, ALU.subtract
        )
        sd3 = work.tile([H, nl], F32, tag=f"sd{l0}")
        nc.scalar.activation(sd3[:], var3[:], AF.Sqrt, bias=cp[:H, C_EPS : C_EPS + 1])
        rstd3 = work.tile([H, nl], F32, tag=f"rstd{l0}")
        nc.vector.reciprocal(rstd3[:], sd3[:])
        gr3 = work.tile([H, nl], F32, tag=f"gr{l0}")
        nc.vector.tensor_tensor(gr3[:], rstd3[:], cp[:H, C_SBG + l0 : C_SBG + l0 + nl], ALU.mult)
        mg3 = work.tile([H, nl], F32, tag=f"mg{l0}")
        nc.vector.tensor_tensor(mg3[:], mu3[:], gr3[:], ALU.mult)
        bf3 = work.tile([H, nl], F32, tag=f"bf{l0}")
        nc.vector.tensor_tensor(bf3[:], cp[:H, C_SBB + l0 : C_SBB + l0 + nl], mg3[:], ALU.subtract)
        zaff = work.tile([H, LBn], F32, tag=f"zaff{l0}")
        grb = gr3[:].unsqueeze(2).broadcast_to([H, nl, B])
        bfb = bf3[:].unsqueeze(2).broadcast_to([H, nl, B])
        zav = zaff[:].rearrange("h (l b) -> h l b", b=B)
        nc.vector.tensor_tensor(zav, z1sb[:].rearrange("h (l b) -> h l b", b=B), grb, ALU.mult)
        nc.vector.tensor_tensor(zav, zav, bfb, ALU.add)
        e = work.tile([H, LBn], F32, tag=f"e{l0}")
        nc.scalar.activation(e[:], zaff[:], AF.Gelu)
        a_ps = psum.tile([H, LBn], F32, tag="mm2")
        for j, li in enumerate(lis):
            nc.tensor.matmul(
                a_ps[:, j * B : (j + 1) * B],
                cp[:H, C_SAW + li * H : C_SAW + (li + 1) * H],
                e[:, j * B : (j + 1) * B],
            )
        az = work.tile([H, LBn], F32, tag=f"az{l0}")
        sabb = cp[:H, C_SAB + l0 : C_SAB + l0 + nl].unsqueeze(2).broadcast_to([H, nl, B])
        nc.vector.tensor_tensor(az[:].rearrange("h (l b) -> h l b", b=B),
                                a_ps[:].rearrange("h (l b) -> h l b", b=B), sabb, ALU.add)
        att = work.tile([H, LBn], F32, tag=f"att{l0}")
        nc.scalar.activation(att[:], az[:], AF.Sigmoid)
        nc.vector.tensor_tensor(
            serocat[:, l0 * B : (l0 + nl) * B], rcat[:, l0 * B : (l0 + nl) * B],
            att[:], ALU.mult,
        )

    sero_block([0, 1])
    f1a_ps = psum.tile([FC[0], B], F32, tag="mm")
    for li in range(2):
        nc.tensor.matmul(
            f1a_ps[:], cp[:H, C_FCW0 + li * H : C_FCW0 + (li + 1) * H],
            serocat[:, li * B : (li + 1) * B],
            start=(li == 0), stop=(li == 1),
        )
    f1a = work.tile([FC[0], B], F32, tag="f1a")
    nc.scalar.copy(f1a[:], f1a_ps[:])

    rg2 = dram.tile([NCORES, BL, H], F32, tag="rg2")
    nc.gpsimd.collective_compute(
        "AllGather",
        ALU.bypass,
        replica_groups=[list(range(NCORES))],
        ins=[rloc2[:].opt()],
        outs=[rg2[:].opt()],
    )
    nc.sync.dma_start(
        rgm[:, 2 * H : 3 * H], rg2[:].rearrange("c l h -> (c l) h")
    )
    sero_block([2])
    seroTs = [serocat[:, li * B : (li + 1) * B] for li in range(N_LAYERS)]

    # ---- FC head (feature-major, BN fused) ----
    def bn_fused(zin_act, F, gcol, bcol, out, relu_bias):
        # z = relu(zin + bias) on DVE; BN stats; affine folded into one stt
        z = work.tile([F, B], F32, tag=f"fcz{F}")
        nc.vector.tensor_scalar(z[:], zin_act[:], relu_bias, 0.0, ALU.add, ALU.max)
        musum = work.tile([F, 1], F32, tag=f"fmus{F}")
        nc.vector.tensor_reduce(musum[:], z[:], AX.X, ALU.add)
        sqs = work.tile([F, B], F32, tag="fsqs")
        nc.vector.tensor_tensor(sqs[:], z[:], z[:], ALU.mult)
        sumsq = work.tile([F, 1], F32, tag=f"fssq{F}")
        nc.vector.tensor_reduce(sumsq[:], sqs[:], AX.X, ALU.add)
        mu = work.tile([F, 1], F32, tag=f"fmu{F}")
        nc.vector.tensor_scalar_mul(mu[:], musum[:], 1.0 / B)
        musq = work.tile([F, 1], F32, tag=f"fmusq{F}")
        nc.vector.tensor_tensor(musq[:], mu[:], mu[:], ALU.mult)
        var = work.tile([F, 1], F32, tag=f"fvar{F}")
        nc.vector.scalar_tensor_tensor(
            var[:], sumsq[:], 1.0 / B, musq[:], ALU.mult, ALU.subtract
        )
        sd = work.tile([F, 1], F32, tag=f"fsd{F}")
        nc.scalar.activation(sd[:], var[:], AF.Sqrt, bias=cp[:F, C_EPS : C_EPS + 1])
        rstd = work.tile([F, 1], F32, tag=f"frstd{F}")
        nc.vector.reciprocal(rstd[:], sd[:])
        gr = work.tile([F, 1], F32, tag=f"fgr{F}")
        nc.vector.tensor_tensor(gr[:], rstd[:], gcol, ALU.mult)
        mg = work.tile([F, 1], F32, tag=f"fmg{F}")
        nc.vector.tensor_tensor(mg[:], mu[:], gr[:], ALU.mult)
        bf = work.tile([F, 1], F32, tag=f"fbf{F}")
        nc.vector.tensor_tensor(bf[:], bcol, mg[:], ALU.subtract)
        nc.vector.scalar_tensor_tensor(
            out[:], z[:], gr[:], bf[:].broadcast_to([F, B]), ALU.mult, ALU.add
        )

    f1b_ps = psum.tile([FC[0], B], F32, tag="mm")
    nc.tensor.matmul(
        f1b_ps[:], cp[:H, C_FCW0 + 2 * H : C_FCW0 + 3 * H], seroTs[2]
    )
    f1sum = work.tile([FC[0], B], F32, tag="f1sum")
    nc.vector.scalar_tensor_tensor(
        f1sum[:], f1b_ps[:], cp[: FC[0], C_FCB0 : C_FCB0 + 1], f1a[:],
        ALU.add, ALU.add,
    )
    z1n = work.tile([FC[0], B], F32, tag="z1n")
    bn_fused(f1sum, FC[0], cp[: FC[0], C_BNG0 : C_BNG0 + 1],
             cp[: FC[0], C_BNB0 : C_BNB0 + 1], z1n, 0.0)
    f2_ps = psum.tile([FC[1], B], F32, tag="mm2")
    nc.tensor.matmul(f2_ps[:], cp[: FC[0], C_FCW1 : C_FCW1 + FC[1]], z1n[:])
    z2n = work.tile([FC[1], B], F32, tag="z2n")
    bn_fused(f2_ps, FC[1], cp[: FC[1], C_BNG1 : C_BNG1 + 1],
             cp[: FC[1], C_BNB1 : C_BNB1 + 1], z2n,
             cp[: FC[1], C_FCB1 : C_FCB1 + 1])
    fo_ps = psum.tile([NCLASS, B], F32, tag="sm")
    nc.tensor.matmul(fo_ps[:], cp[: FC[1], C_FW : C_FW + NCLASS], z2n[:])
    outT = work.tile([NCLASS, B], F32, tag="outT")
    nc.vector.tensor_scalar(
        outT[:], fo_ps[:], cp[:NCLASS, C_FB : C_FB + 1], 0.0, ALU.add, ALU.max
    )
    ot_ps = psum.tile([B, NCLASS], F32, tag="tp")
    nc.tensor.transpose(ot_ps[:], outT[:], ident[:NCLASS, :NCLASS])
    ofin = work.tile([B, NCLASS], F32, tag="ofin")
    nc.vector.tensor_copy(ofin[:], ot_ps[:])
    nc.sync.dma_start(io["out"][:], ofin[:])


def _build():
    nc = bacc.Bacc("TRN2", target_bir_lowering=False, debug=False, num_devices=NCORES)
    io = {}

    def dparam(name, shape, dtype=F32, kind="ExternalInput"):
        io[name] = nc.dram_tensor(name, list(shape), dtype, kind=kind).ap()

    dparam("cpack", (128, C_W))
    dparam("w2pack", (128, 3 * KE * H))
    dparam("xpack", (R, BL * GW))
    dparam("adjpack", (R, BL * R))
    dparam("adjTpack", (R, BL * R))
    dparam("posTpack", (R, BL * R))
    dparam("out", (B, NCLASS), kind="ExternalOutput")

    import contextlib

    with tile.TileContext(nc) as tc:
        with contextlib.ExitStack() as ctx:
            io["consts_pool"] = ctx.enter_context(tc.tile_pool(name="consts", bufs=1))
            io["state_pool"] = ctx.enter_context(tc.tile_pool(name="state", bufs=1))
            io["work_pool"] = ctx.enter_context(tc.tile_pool(name="work", bufs=3))
            io["psum_pool"] = ctx.enter_context(
                tc.tile_pool(name="psum", bufs=2, space="PSUM")
            )
            io["dram_pool"] = ctx.enter_context(
                tc.tile_pool(name="dram", bufs=1, space="DRAM")
            )
            _emit(tc, io)
    nc.compile()
    return nc


def _prep_shared(inputs):
    f = np.float32
    cp = np.zeros((128, C_W), f)
    cp[:, C_ID : C_ID + 128] = np.eye(128, dtype=f)
    cp[:, C_NOTI : C_NOTI + 128] = 1.0 - np.eye(128, dtype=f)
    for i in range(N_LAYERS):
        pw = np.asarray(inputs[f"pw_{i}"], f)
        cp[:, C_PW + i * H : C_PW + i * H + H] = pw / np.linalg.norm(pw)
        cp[:D, C_W1 + i * K : C_W1 + (i + 1) * K] = np.asarray(inputs[f"w1_{i}"], f)
        cp[:H, C_SEW + i * H : C_SEW + (i + 1) * H] = np.asarray(inputs[f"sew_{i}"], f)
        cp[:H, C_SAW + i * H : C_SAW + (i + 1) * H] = np.asarray(inputs[f"saw_{i}"], f)
        cp[:H, C_SAB + i] = np.asarray(inputs[f"sab_{i}"], f)
        cp[:H, C_SBG + i] = np.asarray(inputs[f"sbg_{i}"], f)
        cp[:H, C_SBB + i] = np.asarray(inputs[f"sbb_{i}"], f)
    # fcw_0 [192, 64] -> [64, 3*64]: chunk li holds fcw_0[li*64:(li+1)*64, :]
    cp[:H, C_FCW0 : C_FCW0 + N_LAYERS * FC[0]] = (
        np.asarray(inputs["fcw_0"], f).reshape(N_LAYERS, H, FC[0])
        .transpose(1, 0, 2).reshape(H, N_LAYERS * FC[0])
    )
    cp[: FC[0], C_FCW1 : C_FCW1 + FC[1]] = np.asarray(inputs["fcw_1"], f)
    cp[: FC[1], C_FW : C_FW + NCLASS] = np.asarray(inputs["fw"], f)
    cp[:, C_ONES] = 1.0
    cp[: FC[0], C_FCB0] = np.asarray(inputs["fcb_0"], f)
    cp[: FC[0], C_BNG0] = np.asarray(inputs["bng_0"], f)
    cp[: FC[0], C_BNB0] = np.asarray(inputs["bnb_0"], f)
    cp[: FC[1], C_FCB1] = np.asarray(inputs["fcb_1"], f)
    cp[: FC[1], C_BNG1] = np.asarray(inputs["bng_1"], f)
    cp[: FC[1], C_BNB1] = np.asarray(inputs["bnb_1"], f)
    cp[:NCLASS, C_FB] = np.asarray(inputs["fb"], f)
    cp[:, C_EPS] = EPS_BN
    cp[:, C_ID2 : C_ID2 + 128] = 2.0 * np.eye(128, dtype=f)

    w2p = np.zeros((128, 3 * KE * H), f)
    for i in range(N_LAYERS):
        w2r = np.asarray(inputs[f"w2_{i}"], f).reshape(K, IN[i], HID[i])
        b2r = np.asarray(inputs[f"b2_{i}"], f).reshape(1, IN[i], HID[i])
        w2e = np.concatenate([w2r, b2r], 0).transpose(1, 0, 2).reshape(IN[i], KE * HID[i])
        w2p[: IN[i], i * KE * H : (i + 1) * KE * H] = w2e
    return {"cpack": cp, "w2pack": w2p}


def kernel(**inputs):
    inputs = {k: np.asarray(v) for k, v in inputs.items()}
    if "nc" not in _CACHE:
        _CACHE["nc"] = _build()
    nc = _CACHE["nc"]

    sh = _prep_shared(inputs)
    f = np.float32
    x = np.asarray(inputs["x"], f)
    adj = np.asarray(inputs["adj"], f)
    pos = np.asarray(inputs["pos"], f)
    in_maps = []
    for c in range(NCORES):
        m = dict(sh)
        s = slice(c * BL, (c + 1) * BL)
        xp = np.ones((R, BL * GW), f)
        xs = x[s]
        for g in range(BL):
            xp[:, g * GW : g * GW + D] = xs[g]
        m["xpack"] = xp
        m["adjpack"] = np.ascontiguousarray(
            adj[s].transpose(1, 0, 2).reshape(R, BL * R)
        )
        m["adjTpack"] = np.ascontiguousarray(
            adj[s].transpose(2, 0, 1).reshape(R, BL * R)
        )
        m["posTpack"] = np.ascontiguousarray(
            pos[s].transpose(2, 0, 1).reshape(R, BL * R)
        )
        in_maps.append(m)

    res = run_bass_kernel_spmd(
        nc, in_maps, core_ids=list(range(NCORES)), trace=TRACE
    )
    _CACHE["last_results"] = res
    return res.results[0]["out"]


# revision 36
# speedup vs baseline: 1.0441x; 1.0441x over previous
"""Trainium2 Bass kernel for nn_CustomNetworkGINSeroMean (GIN message passing +
TopK pooling + SERO readout + BN/FC head).

Strategy (data-parallel over batch B=64, 8 graphs per NeuronCore):
  - Inputs arrive as a handful of large packed DMAs (consts in one [128,W]
    tensor; x/adj/adjT/posT packed [100, 8*w] graph-major); host pre-computes
    the adj/pos transposes so the mean aggregation needs none on-chip.
  - U = relu(pos@w1) depends only on the static pos, so all three layers of
    U are computed up front, overlapped with input DMA and the init barrier.
  - Per layer: degrees come from one PE matmul column per graph (deg =
    adjT.T @ alive, masking dead cols for free) + one batched reciprocal;
    h = x + invd*(adj@x) via shared-PSUM y matmuls and broadcast-view DVE
    ops; Wn = (relu(pos@w1)@w2+b2) is never materialized: x_out =
    sum_k U_k*(h@W2r[k]) with U_8 = 1 folding b2 in (f32r matmuls, DVE
    mult+strided-reduce); topk scores batch into one mult + one reduce.
  - TopK pooling is dense with alive-masks (DVE max8/match_replace idiom).
  - augment_adj: half-masked operands make the contraction fully alive-masked
    (alive_j on both sides), M^2 = G + 2*Abar via a PSUM-accumulated 2I
    matmul (+diag, killed by notI); adjT is re-derived by PE transpose.
    Dead-row/col pollution never reaches live math.
  - The r readout is AllGathered in two pieces: layers 0-1 right after layer
    1 (hidden under layer-2 compute, also absorbing cross-core skew) and
    layer 2 at the end; the (0,1) SERO block is emitted before the second
    collective so its sync-queue fence cannot stall that work, overlapping
    it with the gather. BN affine+activations are fused/batched; batch-stat
    BN head is computed redundantly on every core.
"""

import numpy as np

import concourse.bass as bass
import concourse.tile as tile
from concourse import bacc, mybir
from concourse.bass import ts
from concourse.bass_utils import run_bass_kernel_spmd
from concourse.masks import make_identity

F32 = mybir.dt.float32
F32R = mybir.dt.float32r
AF = mybir.ActivationFunctionType
ALU = mybir.AluOpType
AX = mybir.AxisListType

B, R, D = 64, 100, 100
H = 64
K = 8
KE = K + 1
HID = (H, H, H)
IN = (D, H, H)
FC = (64, 32)
NCLASS = 2
N_LAYERS = 3
NCORES = 8
BL = B // NCORES
MS = (50, 25, 13)
NEG = -1.0e30
EPS_BN = 1e-5
GW = R + 1  # per-graph x block width (feats + alive col)

# const pack column offsets
C_ID, C_NOTI, C_PW, C_W1 = 0, 128, 256, 448
C_SEW, C_SAW, C_FCW0, C_FCW1 = 472, 664, 856, 1048
C_FW, C_ONES, C_SAB, C_SBG, C_SBB = 1080, 1082, 1083, 1086, 1089
C_FCB0, C_BNG0, C_BNB0, C_FCB1, C_BNG1, C_BNB1, C_FB = 1092, 1093, 1094, 1095, 1096, 1097, 1098
C_EPS = 1099
C_ID2 = 1100
C_W = 1228

TRACE = False
_CACHE = {}


def _emit(tc, io):
    nc = tc.nc
    consts = io["consts_pool"]
    state = io["state_pool"]
    work = io["work_pool"]
    psum = io["psum_pool"]
    dram = io["dram_pool"]

    cp = consts.tile([128, C_W], F32, tag="cpack")
    nc.sync.dma_start(cp[:], io["cpack"][:])
    w2f = consts.tile([128, 3 * KE * H], F32, tag="w2packf")
    nc.sync.dma_start(w2f[:], io["w2pack"][:])
    w2 = consts.tile([128, 3 * KE * H], F32R, tag="w2pack")
    nc.vector.tensor_copy(w2[:], w2f[:])
    id2r = consts.tile([128, 128], F32R, tag="id2r")
    nc.vector.tensor_copy(id2r[:], cp[:, C_ID2 : C_ID2 + 128])

    xall = state.tile([R, BL * GW], F32, tag="xall")
    nc.sync.dma_start(xall[:], io["xpack"][:])
    adjall = state.tile([R, BL * R], F32, tag="adjall")
    nc.sync.dma_start(adjall[:], io["adjpack"][:])
    adjTall = state.tile([R, BL * R], F32, tag="adjTall")
    nc.sync.dma_start(adjTall[:], io["adjTpack"][:])
    posTall = state.tile([R, BL * R], F32, tag="posTall")
    nc.sync.dma_start(posTall[:], io["posTpack"][:])

    ident = cp[:, C_ID : C_ID + 128]
    notI = cp[:, C_NOTI : C_NOTI + 128]
    ones_col = cp[:, C_ONES : C_ONES + 1]

    # U = relu(pos @ w1) depends only on static pos: compute ALL layers up
    # front (overlaps input DMA / init barrier). Layout [li][g*KE + k], col
    # k=K stays 1.0 to fold the b2 bias in as a 9th term.
    ueall = state.tile([R, N_LAYERS * BL * KE], F32, tag="ueall")
    nc.vector.memset(ueall[:], 1.0)
    for li in range(N_LAYERS):
        ub_ps = psum.tile([R, BL * K], F32, tag="sm")
        for g in range(BL):
            nc.tensor.matmul(
                ub_ps[:, g * K : (g + 1) * K],
                posTall[:, g * R : (g + 1) * R],
                cp[:D, C_W1 + li * K : C_W1 + (li + 1) * K],
            )
        uv = ueall[:, li * BL * KE :].rearrange("n (g e) -> n g e", e=KE)
        nc.scalar.activation(
            uv[:, 0:BL, 0:K], ub_ps[:].rearrange("n (g k) -> n g k", k=K), AF.Relu
        )

    xoall = state.tile([R, BL * H], F32, tag="xoall")
    rlocT = state.tile([BL, N_LAYERS * H], F32, tag="rlocT")


    import os
    ksub = int(os.environ.get("KSUB", "4"))
    kph = int(os.environ.get("KPH", "4"))
    KA = 5  # g1 holds k in [0, KA), g2 holds [KA, KE)
    aliveT = None  # [BL, R] graph-major; None == all alive (layer 0)
    alCol = None

    din = D
    for li in range(N_LAYERS):
        hid = HID[li]
        m_sel = MS[li]
        scoreCol = work.tile([R, BL], F32, tag="scoreCol")

        # ---- batched degrees: deg_g = adjT_g.T @ alive, one reciprocal ----
        deg_ps = psum.tile([R, BL], F32, tag="sm")
        for g in range(BL):
            nc.tensor.matmul(
                deg_ps[:, g : g + 1], adjTall[:, g * R : (g + 1) * R],
                ones_col[:R, :] if alCol is None else alCol[:, g : g + 1],
            )
        invds = work.tile([R, BL], F32, tag="invds")
        nc.vector.reciprocal(invds[:], deg_ps[:])

        # ---- mean aggregation, batched: h = x + invd * (adj @ x) ----
        # y matmuls land in shared PSUM banks (GB graphs per bank), then two
        # broadcast-view DVE ops normalize+add for the whole block at once.
        GB = 4 if din > 64 else 8
        xv = xall[:].rearrange("n (g w) -> n g w", w=GW)
        for b0 in range(0, BL, GB):
            yb_ps = psum.tile([R, GB * din], F32, tag="mm")
            for g in range(b0, b0 + GB):
                nc.tensor.matmul(
                    yb_ps[:, (g - b0) * din : (g - b0 + 1) * din],
                    adjTall[:, g * R : (g + 1) * R], xall[:, g * GW : g * GW + din],
                )
            yv = yb_ps[:].rearrange("n (g e) -> n g e", e=din)
            tb = work.tile([R, GB, din], F32, tag="tb")
            iv = invds[:, b0 : b0 + GB].unsqueeze(2).broadcast_to([R, GB, din])
            nc.vector.tensor_tensor(tb[:], yv, iv, ALU.mult)
            nc.vector.tensor_tensor(
                xv[:, b0 : b0 + GB, 0:din], tb[:], xv[:, b0 : b0 + GB, 0:din],
                ALU.add,
            )
        if kph >= 2:
            # hT = x.T per 4-graph PSUM bank, cast to f32r in two wide copies
            hTall = work.tile([R, BL * R], F32R, tag="hTall")
            for b0 in range(0, BL, 4):
                htb_ps = psum.tile([din, 4 * R], F32, tag="tp")
                for g in range(b0, b0 + 4):
                    nc.tensor.transpose(
                        htb_ps[:, (g - b0) * R : (g - b0 + 1) * R],
                        xall[:, g * GW : g * GW + din], ident[:R, :R],
                    )
                nc.vector.tensor_copy(
                    hTall[0:din, b0 * R : (b0 + 4) * R], htb_ps[:]
                )
            w2r = w2[0:din, li * KE * H : (li + 1) * KE * H]
            for g in range(BL):
                # ---- x_out = sum_k U_k * (h @ W2r[k]) ----
                hT = hTall[0:din, g * R : (g + 1) * R]
                ue = ueall[:, (li * BL + g) * KE : (li * BL + g + 1) * KE]
                g1_ps = psum.tile([R, KA * hid], F32, tag="mm")
                g2_ps = psum.tile([R, (KE - KA) * hid], F32, tag="mm2")
                nc.tensor.matmul(g1_ps[:], hT, w2r[:, 0 : KA * hid])
                nc.tensor.matmul(g2_ps[:], hT, w2r[:, KA * hid : KE * hid])
                if kph < 3:
                    continue
                # DVE: mult+reduce both halves
                pr1 = work.tile([R, hid, KA], F32, tag="pr1")
                g1v = g1_ps[:].rearrange("n (k o) -> n o k", k=KA)
                u1v = ue[:, 0:KA].unsqueeze(1).broadcast_to([R, hid, KA])
                nc.vector.tensor_tensor(pr1[:], g1v, u1v, ALU.mult)
                xo1 = work.tile([R, hid], F32, tag="xo1")
                nc.vector.tensor_reduce(xo1[:], pr1[:], AX.X, ALU.add)
                pr2 = work.tile([R, hid, KE - KA], F32, tag="pr2")
                g2v = g2_ps[:].rearrange("n (k o) -> n o k", k=KE - KA)
                u2v = ue[:, KA:KE].unsqueeze(1).broadcast_to([R, hid, KE - KA])
                nc.vector.tensor_tensor(pr2[:], g2v, u2v, ALU.mult)
                xo2 = work.tile([R, hid], F32, tag="xo2")
                nc.vector.tensor_reduce(xo2[:], pr2[:], AX.X, ALU.add)
                xo = xoall[:, g * H : g * H + hid]
                nc.gpsimd.tensor_tensor(xo, xo1[:], xo2[:], ALU.add)
        if kph >= 4:
            # ---- topk scores, batched: one mult + one strided reduce ----
            sprodall = work.tile([R, BL, hid], F32, tag="sprodall")
            pwb = (
                cp[:R, C_PW + li * H : C_PW + (li + 1) * H]
                .unsqueeze(1).broadcast_to([R, BL, hid])
            )
            nc.vector.tensor_tensor(
                sprodall[:], xoall[:].rearrange("n (g o) -> n g o", o=H), pwb,
                ALU.mult,
            )
            nc.vector.tensor_reduce(scoreCol[:], sprodall[:], AX.X, ALU.add)

        if ksub < 2:
            din = hid
            continue
        # ---- topk (graph-major [BL, R]) ----
        st_ps = psum.tile([BL, R], F32, tag="sm")
        nc.tensor.transpose(st_ps[:], scoreCol[:], ident[:R, :R])
        sm = work.tile([BL, R], F32, tag="smask")
        if aliveT is None:
            nc.vector.tensor_copy(sm[:], st_ps[:])
        else:
            pen = work.tile([BL, R], F32, tag="pen")
            nc.gpsimd.tensor_scalar(pen[:], aliveT[:], -1.0, -NEG, ALU.add, ALU.mult)
            nc.vector.tensor_tensor(sm[:], st_ps[:], aliveT[:], ALU.mult)
            nc.vector.tensor_tensor(sm[:], sm[:], pen[:], ALU.add)
        wk = work.tile([BL, R], F32, tag="wk")
        nc.vector.tensor_copy(wk[:], sm[:])
        nrounds = (m_sel + 7) // 8
        for t in range(nrounds):
            mx = work.tile([BL, 8], F32, tag="mx")
            nc.vector.max(mx[:], wk[:])
            rem = m_sel - 8 * t
            if rem < 8:
                nc.vector.memset(mx[:, rem:8], NEG)
            nc.vector.match_replace(wk[:], mx[:], wk[:], NEG)
        nmT = work.tile([BL, R], F32, tag=f"nmT{li}")
        nc.vector.tensor_tensor(nmT[:], sm[:], wk[:], ALU.subtract)
        nc.vector.tensor_scalar_min(nmT[:], nmT[:], 1.0)
        sig = work.tile([BL, R], F32, tag="sig")
        nc.scalar.activation(sig[:], sm[:], AF.Sigmoid)
        sclT = work.tile([BL, R], F32, tag="sclT")
        nc.vector.tensor_tensor(sclT[:], sig[:], nmT[:], ALU.mult)
        aliveT = nmT
        sc_ps = psum.tile([R, BL], F32, tag="tp")
        nc.tensor.transpose(sc_ps[:], sclT[:], ident[:BL, :BL])
        scalesCol = work.tile([R, BL], F32, tag=f"scales{li}")
        nc.vector.tensor_copy(scalesCol[:], sc_ps[:])
        alCol = work.tile([R, BL], F32, tag=f"alCol{li}")
        nc.gpsimd.tensor_scalar(alCol[:], scalesCol[:], 0.0, None, ALU.is_gt)

        if ksub < 3:
            din = hid
            continue
        # ---- pool x (batched), readout r, augment adj ----
        scb = scalesCol[:].unsqueeze(2).broadcast_to([R, BL, hid])
        nc.vector.tensor_tensor(
            xv[:, :, 0:hid], xoall[:].rearrange("n (g o) -> n g o", o=H), scb,
            ALU.mult,
        )
        rt_ps = psum.tile([H, BL], F32, tag="sm")
        for g in range(BL):
            nc.tensor.matmul(
                rt_ps[:, g : g + 1], xall[:, g * GW : g * GW + hid], ones_col[:R, :]
            )
        rtsb = work.tile([H, BL], F32, tag="rtsb")
        nc.scalar.mul(rtsb[:], rt_ps[:], 1.0 / m_sel)
        rlt_ps = psum.tile([BL, H], F32, tag="tp")
        nc.tensor.transpose(rlt_ps[:], rtsb[:], ident[:H, :H])
        nc.scalar.copy(rlocT[:, ts(li, H)], rlt_ps[:])
        if li == 1:
            rloc01 = dram.tile([BL, 2 * H], F32, tag="rloc01")
            nc.sync.dma_start(rloc01[:], rlocT[:, 0 : 2 * H])
            rg01 = dram.tile([NCORES, BL, 2 * H], F32, tag="rg01")
            nc.gpsimd.collective_compute(
                "AllGather",
                ALU.bypass,
                replica_groups=[list(range(NCORES))],
                ins=[rloc01[:].opt()],
                outs=[rg01[:].opt()],
            )
            io["rg01"] = rg01

        if li < N_LAYERS - 1 and ksub >= 4:
            # masks batched over all graphs via broadcast views
            alb = alCol[:].unsqueeze(2).broadcast_to([R, BL, R])
            adv = adjall[:].rearrange("n (g j) -> n g j", j=R)
            atv = adjTall[:].rearrange("n (g j) -> n g j", j=R)
            ahall = work.tile([R, BL * R], F32R, tag="ahall")
            nc.vector.tensor_tensor(
                ahall[:].rearrange("n (g j) -> n g j", j=R), adv, alb, ALU.mult
            )
            athall = work.tile([R, BL * R], F32R, tag="athall")
            nc.gpsimd.tensor_tensor(
                athall[:].rearrange("n (g j) -> n g j", j=R), atv, alb, ALU.mult
            )
            for g in range(BL):
                a0 = g * R
                G_ps = psum.tile([R, R], F32, tag="mm")
                nc.tensor.matmul(
                    G_ps[:], athall[:, a0 : a0 + R], ahall[:, a0 : a0 + R],
                    start=True, stop=False,
                )
                nc.tensor.matmul(
                    G_ps[:], id2r[:R, :R], ahall[:, a0 : a0 + R],
                    start=False, stop=True,
                )
                nc.vector.tensor_tensor(
                    adjall[:, a0 : a0 + R], G_ps[:], notI[:R, :R], ALU.mult
                )
                at_ps = psum.tile([R, R], F32, tag="mm2")
                nc.tensor.transpose(at_ps[:], adjall[:, a0 : a0 + R], ident[:R, :R])
                nc.scalar.copy(adjTall[:, a0 : a0 + R], at_ps[:])
        din = hid

    import os
    stage = int(os.environ.get("KSTAGE", "3"))
    if stage == 1:
        ofin = work.tile([B, NCLASS], F32, tag="ofin")
        nc.vector.memset(ofin[:], 0.0)
        if kph >= 4:
            nc.vector.tensor_tensor(ofin[:, 0:1], xoall[0:B, 0:1], scoreCol[0:B, 0:1], ALU.add)
        elif kph >= 3:
            nc.scalar.copy(ofin[:, 0:1], xoall[0:B, 0:1])
        nc.sync.dma_start(io["out"][:], ofin[:])
        return

    # ---- layer-2 AllGather is issued AFTER the (0,1) SERO block is
    # emitted, so the collective's sync-queue fence cannot stall the
    # pre-head work that only needs the first gather's result.
    rloc2 = dram.tile([BL, H], F32, tag="rloc2")
    nc.sync.dma_start(rloc2[:], rlocT[:, 2 * H : 3 * H])
    rg01 = io["rg01"]
    rgm = state.tile([B, N_LAYERS * H], F32, tag="rgm")
    nc.sync.dma_start(
        rgm[:].rearrange("cl (l h) -> cl l h", h=H)[:, 0:2, :],
        rg01[:].rearrange("c l (q h) -> (c l) q h", h=H),
    )
    if stage == 2:
        ofin = work.tile([B, NCLASS], F32, tag="ofin")
        nc.vector.memset(ofin[:], 0.0)
        nc.scalar.copy(ofin[:, 0:2], rgm[:, 0:2])
        nc.sync.dma_start(io["out"][:], ofin[:])
        return

    # ---- SERO attention per gather-group: layers (0,1) run while the
    # layer-2 AllGather is still in flight; layer 2 afterward.
    rcat = state.tile([H, N_LAYERS * B], F32, tag="rcat")
    serocat = state.tile([H, N_LAYERS * B], F32, tag="serocat")

    def sero_block(lis):
        nl = len(lis)
        l0 = lis[0]
        for li in lis:
            rt_ps2 = psum.tile([H, B], F32, tag="tp")
            nc.tensor.transpose(rt_ps2[:], rgm[:, li * H : (li + 1) * H], ident[:B, :B])
            nc.scalar.copy(rcat[:, li * B : (li + 1) * B], rt_ps2[:])
        LBn = nl * B
        z1_ps = psum.tile([H, LBn], F32, tag="mm")
        for j, li in enumerate(lis):
            nc.tensor.matmul(
                z1_ps[:, j * B : (j + 1) * B],
                cp[:H, C_SEW + li * H : C_SEW + (li + 1) * H],
                rcat[:, li * B : (li + 1) * B],
            )
        mus = work.tile([H, nl], F32, tag=f"mus{l0}")
        nc.vector.tensor_reduce(mus[:], z1_ps[:].rearrange("h (l b) -> h l b", b=B), AX.X, ALU.add)
        z1sb = work.tile([H, LBn], F32, tag=f"z1sb{l0}")
        nc.scalar.copy(z1sb[:], z1_ps[:])
        sqs = work.tile([H, LBn], F32, tag=f"sqs{l0}")
        nc.vector.tensor_tensor(sqs[:], z1sb[:], z1sb[:], ALU.mult)
        ssq = work.tile([H, nl], F32, tag=f"ssq{l0}")
        nc.vector.tensor_reduce(ssq[:], sqs[:].rearrange("h (l b) -> h l b", b=B), AX.X, ALU.add)
        mu3 = work.tile([H, nl], F32, tag=f"mu{l0}")
        nc.vector.tensor_scalar_mul(mu3[:], mus[:], 1.0 / B)
        musq = work.tile([H, nl], F32, tag=f"musq{l0}")
        nc.vector.tensor_tensor(musq[:], mu3[:], mu3[:], ALU.mult)
        var3 = work.tile([H, nl], F32, tag=f"var{l0}")
        nc.vector.scalar_tensor_tensor(
            var3[:], ssq[:], 1.0 / B, musq[:], ALU.mult, ALU.subtract
        )
        sd3 = work.tile([H, nl], F32, tag=f"sd{l0}")
        nc.scalar.activation(sd3[:], var3[:], AF.Sqrt, bias=cp[:H, C_EPS : C_EPS + 1])
        rstd3 = work.tile([H, nl], F32, tag=f"rstd{l0}")
        nc.vector.reciprocal(rstd3[:], sd3[:])
        gr3 = work.tile([H, nl], F32, tag=f"gr{l0}")
        nc.vector.tensor_tensor(gr3[:], rstd3[:], cp[:H, C_SBG + l0 : C_SBG + l0 + nl], ALU.mult)
        mg3 = work.tile([H, nl], F32, tag=f"mg{l0}")
        nc.vector.tensor_tensor(mg3[:], mu3[:], gr3[:], ALU.mult)
        bf3 = work.tile([H, nl], F32, tag=f"bf{l0}")
        nc.vector.tensor_tensor(bf3[:], cp[:H, C_SBB + l0 : C_SBB + l0 + nl], mg3[:], ALU.subtract)
        zaff = work.tile([H, LBn], F32, tag=f"zaff{l0}")
        grb = gr3[:].unsqueeze(2).broadcast_to([H, nl, B])
        bfb = bf3[:].unsqueeze(2).broadcast_to([H, nl, B])
        zav = zaff[:].rearrange("h (l b) -> h l b", b=B)
        nc.vector.tensor_tensor(zav, z1sb[:].rearrange("h (l b) -> h l b", b=B), grb, ALU.mult)
        nc.vector.tensor_tensor(zav, zav, bfb, ALU.add)
        e = work.tile([H, LBn], F32, tag=f"e{l0}")
        nc.scalar.activation(e[:], zaff[:], AF.Gelu)
        a_ps = psum.tile([H, LBn], F32, tag="mm2")
        for j, li in enumerate(lis):
            nc.tensor.matmul(
                a_ps[:, j * B : (j + 1) * B],
                cp[:H, C_SAW + li * H : C_SAW + (li + 1) * H],
                e[:, j * B : (j + 1) * B],
            )
        az = work.tile([H, LBn], F32, tag=f"az{l0}")
        sabb = cp[:H, C_SAB + l0 : C_SAB + l0 + nl].unsqueeze(2).broadcast_to([H, nl, B])
        nc.vector.tensor_tensor(az[:].rearrange("h (l b) -> h l b", b=B),
                                a_ps[:].rearrange("h (l b) -> h l b", b=B), sabb, ALU.add)
        att = work.tile([H, LBn], F32, tag=f"att{l0}")
        nc.scalar.activation(att[:], az[:], AF.Sigmoid)
        nc.vector.tensor_tensor(
            serocat[:, l0 * B : (l0 + nl) * B], rcat[:, l0 * B : (l0 + nl) * B],
            att[:], ALU.mult,
        )

    sero_block([0, 1])
    f1a_ps = psum.tile([FC[0], B], F32, tag="mm")
    for li in range(2):
        nc.tensor.matmul(
            f1a_ps[:], cp[:H, C_FCW0 + li * H : C_FCW0 + (li + 1) * H],
            serocat[:, li * B : (li + 1) * B],
            start=(li == 0), stop=(li == 1),
        )
    f1a = work.tile([FC[0], B], F32, tag="f1a")
    nc.scalar.copy(f1a[:], f1a_ps[:])

    rg2 = dram.tile([NCORES, BL, H], F32, tag="rg2")
    nc.gpsimd.collective_compute(
        "AllGather",
        ALU.bypass,
        replica_groups=[list(range(NCORES))],
        ins=[rloc2[:].opt()],
        outs=[rg2[:].opt()],
    )
    nc.sync.dma_start(
        rgm[:, 2 * H : 3 * H], rg2[:].rearrange("c l h -> (c l) h")
    )
    sero_block([2])
    seroTs = [serocat[:, li * B : (li + 1) * B] for li in range(N_LAYERS)]

    # ---- FC head (feature-major, BN fused) ----
    def bn_fused(zin_act, F, gcol, bcol, out, relu_bias):
        # z = relu(zin + bias) on DVE; BN stats; affine folded into one stt
        z = work.tile([F, B], F32, tag=f"fcz{F}")
        nc.vector.tensor_scalar(z[:], zin_act[:], relu_bias, 0.0, ALU.add, ALU.max)
        musum = work.tile([F, 1], F32, tag=f"fmus{F}")
        nc.vector.tensor_reduce(musum[:], z[:], AX.X, ALU.add)
        sqs = work.tile([F, B], F32, tag="fsqs")
        nc.vector.tensor_tensor(sqs[:], z[:], z[:], ALU.mult)
        sumsq = work.tile([F, 1], F32, tag=f"fssq{F}")
        nc.vector.tensor_reduce(sumsq[:], sqs[:], AX.X, ALU.add)
        mu = work.tile([F, 1], F32, tag=f"fmu{F}")
        nc.vector.tensor_scalar_mul(mu[:], musum[:], 1.0 / B)
        musq = work.tile([F, 1], F32, tag=f"fmusq{F}")
        nc.vector.tensor_tensor(musq[:], mu[:], mu[:], ALU.mult)
        var = work.tile([F, 1], F32, tag=f"fvar{F}")
        nc.vector.scalar_tensor_tensor(
            var[:], sumsq[:], 1.0 / B, musq[:], ALU.mult, ALU.subtract
        )
        sd = work.tile([F, 1], F32, tag=f"fsd{F}")
        nc.scalar.activation(sd[:], var[:], AF.Sqrt, bias=cp[:F, C_EPS : C_EPS + 1])
        rstd = work.tile([F, 1], F32, tag=f"frstd{F}")
        nc.vector.reciprocal(rstd[:], sd[:])
        gr = work.tile([F, 1], F32, tag=f"fgr{F}")
        nc.vector.tensor_tensor(gr[:], rstd[:], gcol, ALU.mult)
        mg = work.tile([F, 1], F32, tag=f"fmg{F}")
        nc.vector.tensor_tensor(mg[:], mu[:], gr[:], ALU.mult)
        bf = work.tile([F, 1], F32, tag=f"fbf{F}")
        nc.vector.tensor_tensor(bf[:], bcol, mg[:], ALU.subtract)
        nc.vector.scalar_tensor_tensor(
            out[:], z[:], gr[:], bf[:].broadcast_to([F, B]), ALU.mult, ALU.add
        )

    f1b_ps = psum.tile([FC[0], B], F32, tag="mm")
    nc.tensor.matmul(
        f1b_ps[:], cp[:H, C_FCW0 + 2 * H : C_FCW0 + 3 * H], seroTs[2]
    )
    f1sum = work.tile([FC[0], B], F32, tag="f1sum")
    nc.vector.scalar_tensor_tensor(
        f1sum[:], f1b_ps[:], cp[: FC[0], C_FCB0 : C_FCB0 + 1], f1a[:],
        ALU.add, ALU.add,
    )
    z1n = work.tile([FC[0], B], F32, tag="z1n")
    bn_fused(f1sum, FC[0], cp[: FC[0], C_BNG0 : C_BNG0 + 1],
             cp[: FC[0], C_BNB0 : C_BNB0 + 1], z1n, 0.0)
    f2_ps = psum.tile([FC[1], B], F32, tag="mm2")
    nc.tensor.matmul(f2_ps[:], cp[: FC[0], C_FCW1 : C_FCW1 + FC[1]], z1n[:])
    z2n = work.tile([FC[1], B], F32, tag="z2n")
    bn_fused(f2_ps, FC[1], cp[: FC[1], C_BNG1 : C_BNG1 + 1],
             cp[: FC[1], C_BNB1 : C_BNB1 + 1], z2n,
             cp[: FC[1], C_FCB1 : C_FCB1 + 1])
    fo_ps = psum.tile([NCLASS, B], F32, tag="sm")
    nc.tensor.matmul(fo_ps[:], cp[: FC[1], C_FW : C_FW + NCLASS], z2n[:])
    outT = work.tile([NCLASS, B], F32, tag="outT")
    nc.vector.tensor_scalar(
        outT[:], fo_ps[:], cp[:NCLASS, C_FB : C_FB + 1], 0.0, ALU.add, ALU.max
    )
    ot_ps = psum.tile([B, NCLASS], F32, tag="tp")
    nc.tensor.transpose(ot_ps[:], outT[:], ident[:NCLASS, :NCLASS])
    ofin = work.tile([B, NCLASS], F32, tag="ofin")
    nc.vector.tensor_copy(ofin[:], ot_ps[:])
    nc.sync.dma_start(io["out"][:], ofin[:])


def _build():
    nc = bacc.Bacc("TRN2", target_bir_lowering=False, debug=False, num_devices=NCORES)
    io = {}

    def dparam(name, shape, dtype=F32, kind="ExternalInput"):
        io[name] = nc.dram_tensor(name, list(shape), dtype, kind=kind).ap()

    dparam("cpack", (128, C_W))
    dparam("w2pack", (128, 3 * KE * H))
    dparam("xpack", (R, BL * GW))
    dparam("adjpack", (R, BL * R))
    dparam("adjTpack", (R, BL * R))
    dparam("posTpack", (R, BL * R))
    dparam("out", (B, NCLASS), kind="ExternalOutput")

    import contextlib

    with tile.TileContext(nc) as tc:
        with contextlib.ExitStack() as ctx:
            io["consts_pool"] = ctx.enter_context(tc.tile_pool(name="consts", bufs=1))
            io["state_pool"] = ctx.enter_context(tc.tile_pool(name="state", bufs=1))
            io["work_pool"] = ctx.enter_context(tc.tile_pool(name="work", bufs=3))
            io["psum_pool"] = ctx.enter_context(
                tc.tile_pool(name="psum", bufs=2, space="PSUM")
            )
            io["dram_pool"] = ctx.enter_context(
                tc.tile_pool(name="dram", bufs=1, space="DRAM")
            )
            _emit(tc, io)
    nc.compile()
    return nc


def _prep_shared(inputs):
    f = np.float32
    cp = np.zeros((128, C_W), f)
    cp[:, C_ID : C_ID + 128] = np.eye(128, dtype=f)
    cp[:, C_NOTI : C_NOTI + 128] = 1.0 - np.eye(128, dtype=f)
    for i in range(N_LAYERS):
        pw = np.asarray(inputs[f"pw_{i}"], f)
        cp[:, C_PW + i * H : C_PW + i * H + H] = pw / np.linalg.norm(pw)
        cp[:D, C_W1 + i * K : C_W1 + (i + 1) * K] = np.asarray(inputs[f"w1_{i}"], f)
        cp[:H, C_SEW + i * H : C_SEW + (i + 1) * H] = np.asarray(inputs[f"sew_{i}"], f)
        cp[:H, C_SAW + i * H : C_SAW + (i + 1) * H] = np.asarray(inputs[f"saw_{i}"], f)
        cp[:H, C_SAB + i] = np.asarray(inputs[f"sab_{i}"], f)
        cp[:H, C_SBG + i] = np.asarray(inputs[f"sbg_{i}"], f)
        cp[:H, C_SBB + i] = np.asarray(inputs[f"sbb_{i}"], f)
    # fcw_0 [192, 64] -> [64, 3*64]: chunk li holds fcw_0[li*64:(li+1)*64, :]
    cp[:H, C_FCW0 : C_FCW0 + N_LAYERS * FC[0]] = (
        np.asarray(inputs["fcw_0"], f).reshape(N_LAYERS, H, FC[0])
        .transpose(1, 0, 2).reshape(H, N_LAYERS * FC[0])
    )
    cp[: FC[0], C_FCW1 : C_FCW1 + FC[1]] = np.asarray(inputs["fcw_1"], f)
    cp[: FC[1], C_FW : C_FW + NCLASS] = np.asarray(inputs["fw"], f)
    cp[:, C_ONES] = 1.0
    cp[: FC[0], C_FCB0] = np.asarray(inputs["fcb_0"], f)
    cp[: FC[0], C_BNG0] = np.asarray(inputs["bng_0"], f)
    cp[: FC[0], C_BNB0] = np.asarray(inputs["bnb_0"], f)
    cp[: FC[1], C_FCB1] = np.asarray(inputs["fcb_1"], f)
    cp[: FC[1], C_BNG1] = np.asarray(inputs["bng_1"], f)
    cp[: FC[1], C_BNB1] = np.asarray(inputs["bnb_1"], f)
    cp[:NCLASS, C_FB] = np.asarray(inputs["fb"], f)
    cp[:, C_EPS] = EPS_BN
    cp[:, C_ID2 : C_ID2 + 128] = 2.0 * np.eye(128, dtype=f)

    w2p = np.zeros((128, 3 * KE * H), f)
    for i in range(N_LAYERS):
        w2r = np.asarray(inputs[f"w2_{i}"], f).reshape(K, IN[i], HID[i])
        b2r = np.asarray(inputs[f"b2_{i}"], f).reshape(1, IN[i], HID[i])
        w2e = np.concatenate([w2r, b2r], 0).transpose(1, 0, 2).reshape(IN[i], KE * HID[i])
        w2p[: IN[i], i * KE * H : (i + 1) * KE * H] = w2e
    return {"cpack": cp, "w2pack": w2p}


def kernel(**inputs):
    inputs = {k: np.asarray(v) for k, v in inputs.items()}
    if "nc" not in _CACHE:
        _CACHE["nc"] = _build()
    nc = _CACHE["nc"]

    sh = _prep_shared(inputs)
    f = np.float32
    x = np.asarray(inputs["x"], f)
    adj = np.asarray(inputs["adj"], f)
    pos = np.asarray(inputs["pos"], f)
    in_maps = []
    for c in range(NCORES):
        m = dict(sh)
        s = slice(c * BL, (c + 1) * BL)
        xp = np.ones((R, BL * GW), f)
        xs = x[s]
        for g in range(BL):
            xp[:, g * GW : g * GW + D] = xs[g]
        m["xpack"] = xp
        m["adjpack"] = np.ascontiguousarray(
            adj[s].transpose(1, 0, 2).reshape(R, BL * R)
        )
        m["adjTpack"] = np.ascontiguousarray(
            adj[s].transpose(2, 0, 1).reshape(R, BL * R)
        )
        m["posTpack"] = np.ascontiguousarray(
            pos[s].transpose(2, 0, 1).reshape(R, BL * R)
        )
        in_maps.append(m)

    res = run_bass_kernel_spmd(
        nc, in_maps, core_ids=list(range(NCORES)), trace=TRACE
    )
    _CACHE["last_results"] = res
    return res.results[0]["out"]
